# revision 11
# baseline (speedup 1.0000x reference)
"""HGNN 2-layer message-passing kernel for 8 TRN2 NeuronCores.

Math (reference, all f32):
    A = diag(Dv) @ H @ diag(W*De)            [N, E]
    prop(I) = A @ (H^T @ (Dv[:,None]*I))
    X1  = BN(leaky_relu(prop(X @ th1)))      (BN over nodes, train stats)
    out = sigmoid(prop(X1 @ th2)).squeeze()

Device formulation: with G = Dv[:,None]*H,
    prop(I) = G @ ((W*De)[:,None] * (G^T @ I))
so Dv never appears on device. Nodes are sharded across 8 cores
(1024 rows each). Per layer, the G^T@(...) contraction over local nodes
produces a partial [E, h] that is AllReduced; the second matmul
consumes only local rows, so its output stays sharded. BN needs one
tiny stats AllReduce; BN + theta2 are folded into a single per-channel
scale w2 = a*th2 and scalar c2 = b . th2 so the normalized X1 is never
materialized.

G ships in fp16 in both [node, edge] and [edge, node] layouts (H is
0/1 so only the Dv scaling is rounded); all big matmuls run fp16 with
f32 PSUM accumulation. AllReduces are f32.
"""

import numpy as np

import concourse.bass as bass
import concourse.bacc as bacc
import concourse.mybir as mybir
import concourse.tile as tile
from concourse.bass_utils import run_bass_kernel_spmd

# Full problem shape (hardcoded per contract).
N_FULL, E, F, HD = 8192, 4096, 128, 64
N_CORES = 8
S = N_FULL // N_CORES          # 1024 local nodes per core
ST = S // 128                  # 8 node tiles
ET = E // 128                  # 32 edge tiles
BN_EPS = 1e-5

F32 = mybir.dt.float32
F16 = mybir.dt.float16
ADD = mybir.AluOpType.add
MULT = mybir.AluOpType.mult
SUB = mybir.AluOpType.subtract


def build_nc():
    nc = bacc.Bacc(
        "TRN2",
        target_bir_lowering=False,
        debug=False,
        enable_asserts=False,
        num_devices=N_CORES,
    )

    # --- kernel I/O (per-core shards) ---
    Hs_d = nc.dram_tensor("Hs", [S, E], F16, kind="ExternalInput")      # G shard
    HsT_d = nc.dram_tensor("HsT", [E, S], F16, kind="ExternalInput")    # G shard, T
    XsT_d = nc.dram_tensor("XsT", [F, S], F16, kind="ExternalInput")    # X shard, T
    th1_d = nc.dram_tensor("th1", [F, HD], F16, kind="ExternalInput")
    t2r_d = nc.dram_tensor("t2r", [1, HD], F32, kind="ExternalInput")   # theta2 row
    gmr_d = nc.dram_tensor("gmr", [1, HD], F32, kind="ExternalInput")
    btr_d = nc.dram_tensor("btr", [1, HD], F32, kind="ExternalInput")
    # W * De_inv, pre-tiled on host: wde[p, t] = (W*De)[t*128+p]
    wde_d = nc.dram_tensor("wde", [128, ET], F32, kind="ExternalInput")
    out_d = nc.dram_tensor("out", [S], F32, kind="ExternalOutput")

    # --- collective bounce buffers (internal DRAM) ---
    rg = [list(range(N_CORES))]
    ar1_in = nc.dram_tensor("ar1_in", [ET, 128, HD], F32)
    ar1_out = nc.dram_tensor("ar1_out", [ET, 128, HD], F32, addr_space="Shared")
    ar2_in = nc.dram_tensor("ar2_in", [1, 2 * HD], F32)
    ar2_out = nc.dram_tensor("ar2_out", [1, 2 * HD], F32, addr_space="Shared")
    ar3_in = nc.dram_tensor("ar3_in", [ET, 128], F32)
    ar3_out = nc.dram_tensor("ar3_out", [ET, 128], F32, addr_space="Shared")

    with tile.TileContext(nc) as tc:
        with (
            tc.tile_pool(name="persist", bufs=1) as pp,
            tc.tile_pool(name="scratch", bufs=3) as sp,
            tc.tile_pool(name="ps", bufs=4, space="PSUM") as ps,
            tc.tile_pool(name="pstat", bufs=1, space="PSUM") as pstat,
        ):
            # persistent SBUF tensors
            Hs = pp.tile([128, ST * E], F16, tag="Hs")        # [node%, t*E+e]
            HsT = pp.tile([128, ET * S], F16, tag="HsT")      # [edge%, t*S+n]
            XsT = pp.tile([128, S], F16, tag="XsT")           # [feat, node]
            th1 = pp.tile([128, HD], F16, tag="th1")
            U = pp.tile([128, ST * HD], F16, tag="U")         # [node%, t*HD+h]
            M1 = pp.tile([128, ET * HD], F32, tag="M1")       # [edge%, t*HD+h]
            M1f = pp.tile([128, ET * HD], F16, tag="M1f")
            X1 = pp.tile([128, ST * HD], F32, tag="X1")       # [node%, t*HD+h]
            wde = pp.tile([128, ET], F32, tag="wde")          # [edge%, t]
            ones = pp.tile([128, 1], F32, tag="ones")
            onesr = pp.tile([1, 128], F32, tag="onesr")
            rows = pp.tile([1, 8 * HD], F32, tag="rows")      # row-vector workspace
            w2b = pp.tile([128, HD], F32, tag="w2b")
            c2b = pp.tile([128, 1], F32, tag="c2b")
            U2 = pp.tile([128, ST], F32, tag="U2")            # [node%, t]
            U2f = pp.tile([128, ST], F16, tag="U2f")
            M2 = pp.tile([128, ET], F32, tag="M2")            # [edge%, t]
            M2f = pp.tile([128, ET], F16, tag="M2f")
            outs = pp.tile([128, ST], F32, tag="outs")        # [node%, t]

            # ---- input DMAs ----
            nc.sync.dma_start(out=XsT[:], in_=XsT_d[:, :])
            nc.sync.dma_start(out=th1[:], in_=th1_d[:, :])
            nc.sync.dma_start(out=wde[:], in_=wde_d[:, :])
            nc.sync.dma_start(out=rows[:, 0:HD], in_=t2r_d[:, :])
            nc.sync.dma_start(out=rows[:, HD:2 * HD], in_=gmr_d[:, :])
            nc.sync.dma_start(out=rows[:, 2 * HD:3 * HD], in_=btr_d[:, :])
            nc.vector.memset(ones[:], 1.0)
            nc.vector.memset(onesr[:], 1.0)

            for k in range(ST):  # 1 MiB per chunk
                nc.sync.dma_start(
                    out=Hs[:, k * E:(k + 1) * E],
                    in_=Hs_d[k * 128:(k + 1) * 128, :],
                )
            for c in range(8):  # 4 edge tiles = 1 MiB per chunk
                j0 = c * 4
                nc.sync.dma_start(
                    out=HsT[:, j0 * S:(j0 + 4) * S].rearrange("p (j n) -> p j n", j=4),
                    in_=HsT_d[j0 * 128:(j0 + 4) * 128, :].rearrange(
                        "(j p) n -> p j n", p=128),
                )

            # ---- U = X @ th1  -> [node, HD] fp16 ----
            for m in range(ST):
                pu = ps.tile([128, HD], F32, tag="mm")
                nc.tensor.matmul(pu[:], XsT[:, m * 128:(m + 1) * 128], th1[:])
                nc.vector.tensor_copy(U[:, m * HD:(m + 1) * HD], pu[:])

            # ---- layer 1 step 1: M1 = (G^T @ U) * WDe  (local partial) ----
            for mt in range(ET):
                pm = ps.tile([128, HD], F32, tag="mm")
                for k in range(ST):
                    nc.tensor.matmul(
                        pm[:],
                        Hs[:, k * E + mt * 128: k * E + (mt + 1) * 128],
                        U[:, k * HD:(k + 1) * HD],
                        start=(k == 0), stop=(k == ST - 1),
                    )
                nc.vector.tensor_scalar_mul(
                    M1[:, mt * HD:(mt + 1) * HD], pm[:], wde[:, mt:mt + 1])

            # ---- AllReduce M1 (1 MiB f32) ----
            nc.sync.dma_start(
                out=ar1_in[:].rearrange("t p h -> p t h"),
                in_=M1[:].rearrange("p (t h) -> p t h", t=ET))
            nc.gpsimd.collective_compute(
                "AllReduce", ADD, replica_groups=rg,
                ins=[ar1_in[:]], outs=[ar1_out[:]])
            nc.sync.dma_start(
                out=M1[:].rearrange("p (t h) -> p t h", t=ET),
                in_=ar1_out[:].rearrange("t p h -> p t h"))
            nc.vector.tensor_copy(M1f[:], M1[:])

            # ---- layer 1 step 2: X1 = lrelu(G @ M1f); stats ----
            stat_sum = pstat.tile([1, HD], F32, tag="ssum")
            stat_sq = pstat.tile([1, HD], F32, tag="ssq")
            for m in range(ST):
                px = ps.tile([128, HD], F32, tag="mm")
                for k in range(ET):
                    nc.tensor.matmul(
                        px[:],
                        HsT[:, k * S + m * 128: k * S + (m + 1) * 128],
                        M1f[:, k * HD:(k + 1) * HD],
                        start=(k == 0), stop=(k == ET - 1),
                    )
                x1s = X1[:, m * HD:(m + 1) * HD]
                # leaky_relu(x) = max(x, 0.01*x)
                lr = sp.tile([128, HD], F32, tag="lr")
                nc.vector.tensor_scalar_mul(lr[:], px[:], 0.01)
                nc.vector.tensor_max(x1s, px[:], lr[:])
                sq = sp.tile([128, HD], F32, tag="sq")
                nc.scalar.square(sq[:], x1s)
                nc.tensor.matmul(
                    stat_sum[:], ones[:], x1s,
                    start=(m == 0), stop=(m == ST - 1))
                nc.tensor.matmul(
                    stat_sq[:], ones[:], sq[:],
                    start=(m == 0), stop=(m == ST - 1))

            # ---- AllReduce stats (512 B) ----
            nc.scalar.copy(rows[:, 3 * HD:4 * HD], stat_sum[:])
            nc.scalar.copy(rows[:, 4 * HD:5 * HD], stat_sq[:])
            nc.sync.dma_start(out=ar2_in[:], in_=rows[:, 3 * HD:5 * HD])
            nc.gpsimd.collective_compute(
                "AllReduce", ADD, replica_groups=rg,
                ins=[ar2_in[:]], outs=[ar2_out[:]])
            nc.sync.dma_start(out=rows[:, 3 * HD:5 * HD], in_=ar2_out[:])

            # ---- BN fold: w2 = gamma/std * th2 ; c2 = (beta - mu*gamma/std) . th2
            t2r = rows[:, 0:HD]
            gmr = rows[:, HD:2 * HD]
            btr = rows[:, 2 * HD:3 * HD]
            gsum = rows[:, 3 * HD:4 * HD]
            gsq = rows[:, 4 * HD:5 * HD]
            mu = rows[:, 5 * HD:6 * HD]
            aa = rows[:, 6 * HD:7 * HD]
            w2r = rows[:, 7 * HD:8 * HD]
            inv_n = 1.0 / float(N_FULL)
            nc.vector.tensor_scalar_mul(mu, gsum, inv_n)
            nc.vector.tensor_scalar_mul(gsq, gsq, inv_n)          # E[x^2]
            nc.scalar.square(aa, mu)                              # mu^2
            nc.vector.tensor_sub(gsq, gsq, aa)                    # var
            nc.vector.tensor_scalar_add(gsq, gsq, BN_EPS)
            nc.scalar.sqrt(gsq, gsq)
            nc.vector.reciprocal(gsq, gsq)                        # 1/std
            nc.vector.tensor_mul(aa, gsq, gmr)                    # a = gamma/std
            nc.vector.tensor_mul(w2r, aa, t2r)                    # w2 = a * th2
            nc.vector.tensor_mul(aa, mu, aa)                      # mu*a
            nc.vector.tensor_sub(aa, btr, aa)                     # b = beta - mu*a
            nc.vector.tensor_mul(aa, aa, t2r)                     # b * th2
            c2r = rows[:, 5 * HD:5 * HD + 1]
            nc.vector.tensor_reduce(c2r, aa, axis=mybir.AxisListType.X, op=ADD)
            pb = ps.tile([128, HD], F32, tag="mm")
            nc.tensor.matmul(pb[:], onesr[:], w2r)
            nc.vector.tensor_copy(w2b[:], pb[:])
            pb2 = ps.tile([128, HD], F32, tag="mm")
            nc.tensor.matmul(pb2[:, 0:1], onesr[:], c2r)
            nc.vector.tensor_copy(c2b[:], pb2[:, 0:1])

            # ---- U2 = X1n @ th2 = X1 @ w2 + c2  -> [node, 1] fp16 ----
            for m in range(ST):
                tmp = sp.tile([128, HD], F32, tag="tmp")
                nc.vector.tensor_mul(tmp[:], X1[:, m * HD:(m + 1) * HD], w2b[:])
                nc.vector.tensor_reduce(
                    U2[:, m:m + 1], tmp[:], axis=mybir.AxisListType.X, op=ADD)
            nc.vector.tensor_scalar(U2f[:], U2[:], c2b[:, 0:1], None, ADD)

            # ---- layer 2 step 1: M2 = (G^T @ U2) * WDe (local partial) ----
            for mt in range(ET):
                pc = ps.tile([128, HD], F32, tag="mm")
                for k in range(ST):
                    nc.tensor.matmul(
                        pc[:, 0:1],
                        Hs[:, k * E + mt * 128: k * E + (mt + 1) * 128],
                        U2f[:, k:k + 1],
                        start=(k == 0), stop=(k == ST - 1),
                    )
                nc.vector.tensor_scalar_mul(
                    M2[:, mt:mt + 1], pc[:, 0:1], wde[:, mt:mt + 1])

            # ---- AllReduce M2 (16 KiB f32) ----
            nc.sync.dma_start(out=ar3_in[:].rearrange("t p -> p t"), in_=M2[:])
            nc.gpsimd.collective_compute(
                "AllReduce", ADD, replica_groups=rg,
                ins=[ar3_in[:]], outs=[ar3_out[:]])
            nc.sync.dma_start(out=M2[:], in_=ar3_out[:].rearrange("t p -> p t"))
            nc.vector.tensor_copy(M2f[:], M2[:])

            # ---- layer 2 step 2: out = sigmoid(G @ M2f) ----
            for m in range(ST):
                po = ps.tile([128, HD], F32, tag="mm")
                for k in range(ET):
                    nc.tensor.matmul(
                        po[:, 0:1],
                        HsT[:, k * S + m * 128: k * S + (m + 1) * 128],
                        M2f[:, k:k + 1],
                        start=(k == 0), stop=(k == ET - 1),
                    )
                nc.scalar.activation(
                    outs[:, m:m + 1], po[:, 0:1],
                    mybir.ActivationFunctionType.Sigmoid)

            nc.sync.dma_start(out=out_d[:].rearrange("(t p) -> p t", p=128),
                              in_=outs[:])

    return nc


_NC = None


def _get_nc():
    global _NC
    if _NC is None:
        _NC = build_nc()
        _NC.finalize()
    return _NC


def make_in_maps(X, Dv_inv, De_inv, H, W, theta1, theta2, gamma, beta):
    X = np.asarray(X, np.float32)
    Dv_inv = np.asarray(Dv_inv, np.float32)
    De_inv = np.asarray(De_inv, np.float32)
    H = np.asarray(H, np.float32)
    W = np.asarray(W, np.float32)
    G = (Dv_inv[:, None] * H).astype(np.float16)
    wde = np.ascontiguousarray((W * De_inv).reshape(ET, 128).T, np.float32)
    th1 = np.ascontiguousarray(theta1, np.float16)
    t2r = np.ascontiguousarray(np.asarray(theta2, np.float32).reshape(1, HD))
    gmr = np.ascontiguousarray(gamma, np.float32).reshape(1, HD)
    btr = np.ascontiguousarray(beta, np.float32).reshape(1, HD)
    in_maps = []
    for i in range(N_CORES):
        sl = slice(i * S, (i + 1) * S)
        Gi = np.ascontiguousarray(G[sl])
        in_maps.append({
            "Hs": Gi,
            "HsT": np.ascontiguousarray(Gi.T),
            "XsT": np.ascontiguousarray(X[sl].T.astype(np.float16)),
            "th1": th1,
            "t2r": t2r,
            "gmr": gmr,
            "btr": btr,
            "wde": wde,
        })
    return in_maps


def kernel(X, Dv_inv, De_inv, H, W, theta1, theta2, gamma, beta):
    nc = _get_nc()
    in_maps = make_in_maps(X, Dv_inv, De_inv, H, W, theta1, theta2, gamma, beta)
    res = run_bass_kernel_spmd(nc, in_maps, core_ids=list(range(N_CORES)))
    return np.concatenate([res.results[i]["out"] for i in range(N_CORES)]).astype(
        np.float32)


# revision 12
# speedup vs baseline: 1.2296x; 1.2296x over previous
"""HGNN 2-layer message-passing kernel for 8 TRN2 NeuronCores.

Math (reference, all f32):
    A = diag(Dv) @ H @ diag(W*De)            [N, E]
    prop(I) = A @ (H^T @ (Dv[:,None]*I))
    X1  = BN(leaky_relu(prop(X @ th1)))      (BN over nodes, train stats)
    out = sigmoid(prop(X1 @ th2)).squeeze()

Device formulation: with G = Dv[:,None]*H,
    prop(I) = G @ ((W*De)[:,None] * (G^T @ I))
so Dv never appears on device. Nodes are sharded across 8 cores
(1024 rows each). Per layer, the G^T@(...) contraction over local nodes
produces a partial [E, h] that is AllReduced; the second matmul
consumes only local rows, so its output stays sharded. BN needs one
tiny stats AllReduce; BN + theta2 are folded into a single per-channel
scale w2 = a*th2 and scalar c2 = b . th2 so the normalized X1 is never
materialized.

G ships in fp16 in both layouts, host-permuted so every DMA descriptor
moves >=16 KiB contiguous per partition. The M1/M2 AllReduces run in
fp16 on DRAM buffers that mirror the SBUF layout (contiguous rows, no
cast, no transpose); the stats AllReduce stays f32. Bounce DMAs ride
the GpSimd SWDGE ring so they never queue behind the H loads on the
sync ring.
"""

import numpy as np

import concourse.bass as bass
import concourse.bacc as bacc
import concourse.mybir as mybir
import concourse.tile as tile
from concourse.bass_utils import run_bass_kernel_spmd

# Full problem shape (hardcoded per contract).
N_FULL, E, F, HD = 8192, 4096, 128, 64
N_CORES = 8
S = N_FULL // N_CORES          # 1024 local nodes per core
ST = S // 128                  # 8 node tiles
ET = E // 128                  # 32 edge tiles
BN_EPS = 1e-5

F32 = mybir.dt.float32
F16 = mybir.dt.float16
ADD = mybir.AluOpType.add
MULT = mybir.AluOpType.mult


def build_nc():
    nc = bacc.Bacc(
        "TRN2",
        target_bir_lowering=False,
        debug=False,
        enable_asserts=False,
        num_devices=N_CORES,
    )

    # --- kernel I/O (per-core shards; H layouts host-permuted) ---
    # Hs_d[p, k, e]  = G[k*128+p, e]   (node-major tiles)
    # HsT_d[p, t, n] = G[n, t*128+p]   (edge-major tiles)
    Hs_d = nc.dram_tensor("Hs", [128, ST, E], F16, kind="ExternalInput")
    HsT_d = nc.dram_tensor("HsT", [128, ET, S], F16, kind="ExternalInput")
    XsT_d = nc.dram_tensor("XsT", [F, S], F16, kind="ExternalInput")
    th1_d = nc.dram_tensor("th1", [F, HD], F16, kind="ExternalInput")
    t2r_d = nc.dram_tensor("t2r", [1, HD], F32, kind="ExternalInput")
    gmr_d = nc.dram_tensor("gmr", [1, HD], F32, kind="ExternalInput")
    btr_d = nc.dram_tensor("btr", [1, HD], F32, kind="ExternalInput")
    # W * De_inv, pre-tiled on host: wde[p, t] = (W*De)[t*128+p]
    wde_d = nc.dram_tensor("wde", [128, ET], F32, kind="ExternalInput")
    out_d = nc.dram_tensor("out", [S], F32, kind="ExternalOutput")

    # --- collective bounce buffers (internal DRAM, SBUF-mirroring layout) ---
    rg = [list(range(N_CORES))]
    ar1_in = nc.dram_tensor("ar1_in", [128, ET * HD], F16)
    ar1_out = nc.dram_tensor("ar1_out", [128, ET * HD], F16, addr_space="Shared")
    ar2_in = nc.dram_tensor("ar2_in", [1, 2 * HD], F32)
    ar2_out = nc.dram_tensor("ar2_out", [1, 2 * HD], F32, addr_space="Shared")
    ar3_in = nc.dram_tensor("ar3_in", [128, ET], F16)
    ar3_out = nc.dram_tensor("ar3_out", [128, ET], F16, addr_space="Shared")

    with tile.TileContext(nc) as tc:
        with (
            tc.tile_pool(name="persist", bufs=1) as pp,
            tc.tile_pool(name="scratch", bufs=3) as sp,
            tc.tile_pool(name="ps", bufs=4, space="PSUM") as ps,
            tc.tile_pool(name="pstat", bufs=1, space="PSUM") as pstat,
        ):
            # persistent SBUF tensors
            Hs = pp.tile([128, ST * E], F16, tag="Hs")        # [node%, k*E+e]
            HsT = pp.tile([128, ET * S], F16, tag="HsT")      # [edge%, t*S+n]
            XsT = pp.tile([128, S], F16, tag="XsT")           # [feat, node]
            th1 = pp.tile([128, HD], F16, tag="th1")
            U = pp.tile([128, ST * HD], F16, tag="U")         # [node%, k*HD+h]
            M1f = pp.tile([128, ET * HD], F16, tag="M1f")     # [edge%, t*HD+h]
            X1 = pp.tile([128, ST * HD], F32, tag="X1")       # [node%, k*HD+h]
            wde = pp.tile([128, ET], F32, tag="wde")          # [edge%, t]
            ones = pp.tile([128, 1], F32, tag="ones")
            onesr = pp.tile([1, 128], F32, tag="onesr")
            rows = pp.tile([1, 8 * HD], F32, tag="rows")      # row workspace
            w2b = pp.tile([128, HD], F32, tag="w2b")
            c2b = pp.tile([128, 1], F32, tag="c2b")
            u2t = pp.tile([128, ST * HD], F32, tag="u2t")     # U2 mul scratch
            U2 = pp.tile([128, ST], F32, tag="U2")            # [node%, k]
            U2f = pp.tile([128, ST], F16, tag="U2f")
            M2f = pp.tile([128, ET], F16, tag="M2f")          # [edge%, t]
            outs = pp.tile([128, ST], F32, tag="outs")        # [node%, k]

            # ---- small input DMAs on the GpSimd (SWDGE) ring ----
            nc.gpsimd.dma_start(out=XsT[:], in_=XsT_d[:, :])
            nc.gpsimd.dma_start(out=th1[:], in_=th1_d[:, :])
            nc.gpsimd.dma_start(out=wde[:], in_=wde_d[:, :])
            nc.gpsimd.dma_start(out=rows[:, 0:HD], in_=t2r_d[:, :])
            nc.gpsimd.dma_start(out=rows[:, HD:2 * HD], in_=gmr_d[:, :])
            nc.gpsimd.dma_start(out=rows[:, 2 * HD:3 * HD], in_=btr_d[:, :])
            nc.vector.memset(ones[:], 1.0)
            nc.vector.memset(onesr[:], 1.0)

            # ---- H loads on the sync (HWDGE) ring: Hs first, 2 MiB chunks,
            # >=16 KiB contiguous per partition ----
            for c in range(ST // 2):
                k0 = c * 2
                nc.sync.dma_start(
                    out=Hs[:, k0 * E:(k0 + 2) * E],
                    in_=Hs_d[:, k0:k0 + 2, :],
                )
            for c in range(ET // 8):
                t0 = c * 8
                nc.sync.dma_start(
                    out=HsT[:, t0 * S:(t0 + 8) * S],
                    in_=HsT_d[:, t0:t0 + 8, :],
                )

            # ---- U = X @ th1  -> [node, HD] fp16 ----
            for m in range(ST):
                pu = ps.tile([128, HD], F32, tag="mm")
                nc.tensor.matmul(pu[:], XsT[:, m * 128:(m + 1) * 128], th1[:])
                nc.vector.tensor_copy(U[:, m * HD:(m + 1) * HD], pu[:])

            # ---- layer 1 step 1: M1 = (G^T @ U) * WDe  (local partial, fp16) ----
            for mt in range(ET):
                pm = ps.tile([128, HD], F32, tag="mm")
                for k in range(ST):
                    nc.tensor.matmul(
                        pm[:],
                        Hs[:, k * E + mt * 128: k * E + (mt + 1) * 128],
                        U[:, k * HD:(k + 1) * HD],
                        start=(k == 0), stop=(k == ST - 1),
                    )
                nc.vector.tensor_scalar_mul(
                    M1f[:, mt * HD:(mt + 1) * HD], pm[:], wde[:, mt:mt + 1])

            # ---- AllReduce M1 (512 KiB fp16, layout = SBUF rows) ----
            nc.gpsimd.dma_start(out=ar1_in[:, :], in_=M1f[:])
            nc.gpsimd.collective_compute(
                "AllReduce", ADD, replica_groups=rg,
                ins=[ar1_in[:]], outs=[ar1_out[:]])
            nc.gpsimd.dma_start(out=M1f[:], in_=ar1_out[:, :])

            # ---- layer 1 step 2: X1 = lrelu(G @ M1f); stats ----
            stat_sum = pstat.tile([1, HD], F32, tag="ssum")
            stat_sq = pstat.tile([1, HD], F32, tag="ssq")
            for m in range(ST):
                px = ps.tile([128, HD], F32, tag="mm")
                for k in range(ET):
                    nc.tensor.matmul(
                        px[:],
                        HsT[:, k * S + m * 128: k * S + (m + 1) * 128],
                        M1f[:, k * HD:(k + 1) * HD],
                        start=(k == 0), stop=(k == ET - 1),
                    )
                x1s = X1[:, m * HD:(m + 1) * HD]
                # leaky_relu(x) = max(x, 0.01*x)
                lr = sp.tile([128, HD], F32, tag="lr")
                nc.vector.tensor_scalar_mul(lr[:], px[:], 0.01)
                nc.vector.tensor_max(x1s, px[:], lr[:])
                sq = sp.tile([128, HD], F32, tag="sq")
                nc.scalar.square(sq[:], x1s)
                nc.tensor.matmul(
                    stat_sum[:], ones[:], x1s,
                    start=(m == 0), stop=(m == ST - 1))
                nc.tensor.matmul(
                    stat_sq[:], ones[:], sq[:],
                    start=(m == 0), stop=(m == ST - 1))

            # ---- AllReduce stats (512 B f32) ----
            nc.scalar.copy(rows[:, 3 * HD:4 * HD], stat_sum[:])
            nc.scalar.copy(rows[:, 4 * HD:5 * HD], stat_sq[:])
            nc.gpsimd.dma_start(out=ar2_in[:], in_=rows[:, 3 * HD:5 * HD])
            nc.gpsimd.collective_compute(
                "AllReduce", ADD, replica_groups=rg,
                ins=[ar2_in[:]], outs=[ar2_out[:]])
            nc.gpsimd.dma_start(out=rows[:, 3 * HD:5 * HD], in_=ar2_out[:])

            # ---- BN fold: w2 = gamma/std * th2 ; c2 = (beta - mu*gamma/std) . th2
            t2r = rows[:, 0:HD]
            gmr = rows[:, HD:2 * HD]
            btr = rows[:, 2 * HD:3 * HD]
            gsum = rows[:, 3 * HD:4 * HD]
            gsq = rows[:, 4 * HD:5 * HD]
            mu = rows[:, 5 * HD:6 * HD]
            aa = rows[:, 6 * HD:7 * HD]
            w2r = rows[:, 7 * HD:8 * HD]
            inv_n = 1.0 / float(N_FULL)
            nc.vector.tensor_scalar_mul(mu, gsum, inv_n)
            nc.vector.tensor_scalar_mul(gsq, gsq, inv_n)          # E[x^2]
            nc.scalar.square(aa, mu)                              # mu^2
            nc.vector.tensor_sub(gsq, gsq, aa)                    # var
            nc.vector.tensor_scalar_add(gsq, gsq, BN_EPS)
            nc.scalar.sqrt(gsq, gsq)
            nc.vector.reciprocal(gsq, gsq)                        # 1/std
            nc.vector.tensor_mul(aa, gsq, gmr)                    # a = gamma/std
            nc.vector.tensor_mul(w2r, aa, t2r)                    # w2 = a * th2
            nc.vector.tensor_mul(aa, mu, aa)                      # mu*a
            nc.vector.tensor_sub(aa, btr, aa)                     # b = beta - mu*a
            nc.vector.tensor_mul(aa, aa, t2r)                     # b * th2
            c2r = rows[:, 5 * HD:5 * HD + 1]
            nc.vector.tensor_reduce(c2r, aa, axis=mybir.AxisListType.X, op=ADD)
            # broadcast w2 / c2 across partitions via K=1 matmul
            pb = ps.tile([128, HD], F32, tag="mm")
            nc.tensor.matmul(pb[:], onesr[:], w2r)
            nc.vector.tensor_copy(w2b[:], pb[:])
            pb2 = ps.tile([128, HD], F32, tag="mm")
            nc.tensor.matmul(pb2[:, 0:1], onesr[:], c2r)
            nc.vector.tensor_copy(c2b[:], pb2[:, 0:1])

            # ---- U2 = X1n @ th2 = X1 @ w2 + c2  -> [node, 1] fp16 ----
            for m in range(ST):
                nc.vector.tensor_mul(
                    u2t[:, m * HD:(m + 1) * HD],
                    X1[:, m * HD:(m + 1) * HD], w2b[:])
            nc.vector.tensor_reduce(
                U2[:], u2t[:].rearrange("p (k h) -> p k h", k=ST),
                axis=mybir.AxisListType.X, op=ADD)
            nc.vector.tensor_scalar(U2f[:], U2[:], c2b[:, 0:1], None, ADD)

            # ---- layer 2 step 1: M2 = (G^T @ U2) * WDe (local partial, fp16) ----
            for g in range(ET // 8):
                pc = ps.tile([128, 8], F32, tag="mm")
                for j in range(8):
                    mt = g * 8 + j
                    for k in range(ST):
                        nc.tensor.matmul(
                            pc[:, j:j + 1],
                            Hs[:, k * E + mt * 128: k * E + (mt + 1) * 128],
                            U2f[:, k:k + 1],
                            start=(k == 0), stop=(k == ST - 1),
                        )
                nc.vector.tensor_mul(
                    M2f[:, g * 8:(g + 1) * 8], pc[:], wde[:, g * 8:(g + 1) * 8])

            # ---- AllReduce M2 (16 KiB fp16) ----
            nc.gpsimd.dma_start(out=ar3_in[:, :], in_=M2f[:])
            nc.gpsimd.collective_compute(
                "AllReduce", ADD, replica_groups=rg,
                ins=[ar3_in[:]], outs=[ar3_out[:]])
            nc.gpsimd.dma_start(out=M2f[:], in_=ar3_out[:, :])

            # ---- layer 2 step 2: out = sigmoid(G @ M2f) ----
            for m in range(ST):
                po = ps.tile([128, 8], F32, tag="mm")
                for k in range(ET):
                    nc.tensor.matmul(
                        po[:, 0:1],
                        HsT[:, k * S + m * 128: k * S + (m + 1) * 128],
                        M2f[:, k:k + 1],
                        start=(k == 0), stop=(k == ET - 1),
                    )
                nc.scalar.activation(
                    outs[:, m:m + 1], po[:, 0:1],
                    mybir.ActivationFunctionType.Sigmoid)

            nc.gpsimd.dma_start(out=out_d[:].rearrange("(t p) -> p t", p=128),
                                in_=outs[:])

    return nc


_NC = None


def _get_nc():
    global _NC
    if _NC is None:
        _NC = build_nc()
        _NC.finalize()
    return _NC


def make_in_maps(X, Dv_inv, De_inv, H, W, theta1, theta2, gamma, beta):
    X = np.asarray(X, np.float32)
    Dv_inv = np.asarray(Dv_inv, np.float32)
    De_inv = np.asarray(De_inv, np.float32)
    H = np.asarray(H, np.float32)
    W = np.asarray(W, np.float32)
    G = (Dv_inv[:, None] * H).astype(np.float16)
    wde = np.ascontiguousarray((W * De_inv).reshape(ET, 128).T, np.float32)
    th1 = np.ascontiguousarray(theta1, np.float16)
    t2r = np.ascontiguousarray(np.asarray(theta2, np.float32).reshape(1, HD))
    gmr = np.ascontiguousarray(gamma, np.float32).reshape(1, HD)
    btr = np.ascontiguousarray(beta, np.float32).reshape(1, HD)
    in_maps = []
    for i in range(N_CORES):
        sl = slice(i * S, (i + 1) * S)
        Gi = G[sl]
        # Hs_host[p, k, e] = Gi[k*128+p, e]
        hs = np.ascontiguousarray(Gi.reshape(ST, 128, E).transpose(1, 0, 2))
        # HsT_host[p, t, n] = Gi[n, t*128+p]
        hst = np.ascontiguousarray(Gi.T.reshape(ET, 128, S).transpose(1, 0, 2))
        in_maps.append({
            "Hs": hs,
            "HsT": hst,
            "XsT": np.ascontiguousarray(X[sl].T.astype(np.float16)),
            "th1": th1,
            "t2r": t2r,
            "gmr": gmr,
            "btr": btr,
            "wde": wde,
        })
    return in_maps


def kernel(X, Dv_inv, De_inv, H, W, theta1, theta2, gamma, beta):
    nc = _get_nc()
    in_maps = make_in_maps(X, Dv_inv, De_inv, H, W, theta1, theta2, gamma, beta)
    res = run_bass_kernel_spmd(nc, in_maps, core_ids=list(range(N_CORES)))
    return np.concatenate([res.results[i]["out"] for i in range(N_CORES)]).astype(
        np.float32)
